# revision 1
# baseline (speedup 1.0000x reference)
"""Causal multi-head self-attention (B=8, S=2048, D=384, H=4, Hd=96) on 8
Trainium2 NeuronCores — v2.

Sharding: data-parallel over batch — each core processes one batch element,
weights replicated. No collectives.

v2 changes vs baseline (engine rebalance, PE trim):
  - exp is the ONLY Activation-engine work during attention; Q/K bias-adds
    (PSUM->SBUF drains) run on ACT during the projection phase when it is
    otherwise idle
  - probabilities (pt) and V' are bf16: halves SBUF traffic, keeps PV at
    1 cyc/col even for narrow diagonal tiles
  - causal trim: QK matmul, exp, and PV all skip columns < rt for diagonal
    k-tiles (per-tile), cutting ~15% of QK matmul cols and ~18% of exp
  - both rank-1 bias matmuls (V' ones/bias row, output bo) are gone: biases
    and the ones-column are folded into the PSUM->SBUF drain ops as
    tensor_tensor adds against broadcast tiles built once at start
  - V' ones column moved to index 0 of each 97-col head block, so the
    softmax denominator lands on PSUM partition 0 (no DMA staging before
    reciprocal/broadcast)
  - denominator drain / reciprocal / normalize on DVE, partition broadcast
    on Pool, diagonal masks on DVE in bf16
"""

import os
import sys

sys.path.insert(0, "/opt/trn_rl_repo")

import numpy as np
import ml_dtypes

import concourse.bass as bass
import concourse.tile as tile
from concourse import bacc, mybir
from concourse.bass_utils import run_bass_kernel_spmd

N_CORES = 8
S = 2048
D = 384
H = 4
HD = 96
CH = 512          # q-chunk width (columns per matmul)
NCH = S // CH     # 4 q-chunks
P = 128           # k-tile height / partition dim
KTN = S // P      # 16 k-tiles
SCALE = 1.0 / np.sqrt(HD)
SHIFT = 2.0   # softmax shift for fp8 chunks (cancels in normalization)

F32 = mybir.dt.float32
BF16 = mybir.dt.bfloat16
F8 = mybir.dt.float8e4
MM_DT = os.environ.get("ATTN_MM_DT", "float32r")  # float32r | float32


def build_nc(repeat=1, variant=(), loop_n=0):
    nc = bacc.Bacc("TRN2", target_bir_lowering=False, debug=False,
                   enable_asserts=False, num_devices=N_CORES)
    MF = mybir.dt.float32r if MM_DT == "float32r" else F32

    xt_d = nc.dram_tensor("xt", [D, S], BF16, kind="ExternalInput").ap()
    wq_d = nc.dram_tensor("wq", [D, D], BF16, kind="ExternalInput").ap()
    wk_d = nc.dram_tensor("wk", [D, D], BF16, kind="ExternalInput").ap()
    wvx_d = nc.dram_tensor("wvx", [D, 97 * H + 31], BF16, kind="ExternalInput").ap()
    wo_d = nc.dram_tensor("wo", [97 * H, D], BF16, kind="ExternalInput").ap()
    bqh_d = nc.dram_tensor("bqh", [HD, H], F32, kind="ExternalInput").ap()
    bkh_d = nc.dram_tensor("bkh", [HD, H], F32, kind="ExternalInput").ap()
    vb_d = nc.dram_tensor("vb", [P, 97 * H + 31], F32, kind="ExternalInput").ap()
    bo_d = nc.dram_tensor("bo", [P, D], F32, kind="ExternalInput").ap()
    ones_d = nc.dram_tensor("onesrow", [1, P], BF16, kind="ExternalInput").ap()
    msk_d = nc.dram_tensor("msk", [P, P], BF16, kind="ExternalInput").ap()
    mskp_d = nc.dram_tensor("mskp", [P, 2, 256], F8, kind="ExternalInput").ap()
    out_d = nc.dram_tensor("out", [S, D], F32, kind="ExternalOutput").ap()
    scr_d = nc.dram_tensor("recscr", [H * NCH, CH], F32, kind="Internal").ap()

    Exp = mybir.ActivationFunctionType.Exp
    mult = mybir.AluOpType.mult
    add = mybir.AluOpType.add

    with tile.TileContext(nc) as tc:
        wpool = tc.alloc_tile_pool(name="w", bufs=1)
        xpool = tc.alloc_tile_pool(name="x", bufs=1)
        qkt_pool = tc.alloc_tile_pool(name="qkt", bufs=1)
        vpool = tc.alloc_tile_pool(name="v", bufs=1)
        ppool = tc.alloc_tile_pool(name="p", bufs=12)
        onpool = tc.alloc_tile_pool(name="on", bufs=2)
        rpool = tc.alloc_tile_pool(name="r", bufs=4)
        qkpool = tc.alloc_tile_pool(name="qkps", bufs=2, space="PSUM")
        accpool = tc.alloc_tile_pool(name="accps", bufs=2, space="PSUM")
        fpool = tc.alloc_tile_pool(name="fps", bufs=2, space="PSUM")

        import contextlib
        loop_ctx = (tc.For_i(0, loop_n, 1) if loop_n
                    else contextlib.nullcontext())
        with loop_ctx:
          for _rep in range(repeat):
            # ---- load weights / constants ----
            xt_sb, wq_sb, wk_sb, wv_sb, wo_sb = [], [], [], [], []
            # weights first, then x chunk-by-chunk so chunk-0 projections
            # start as soon as its quarter of x lands
            for t in range(3):
                wqt = wpool.tile([P, D], BF16, name=f"wq{t}", tag=f"wq{t}")
                nc.sync.dma_start(wqt[:], wq_d[P * t:P * t + P, :])
                wq_sb.append(wqt)
                wkt = wpool.tile([P, D], BF16, name=f"wk{t}", tag=f"wk{t}")
                nc.sync.dma_start(wkt[:], wk_d[P * t:P * t + P, :])
                wk_sb.append(wkt)
                wvt = wpool.tile([P, 97 * H + 31], BF16, name=f"wv{t}", tag=f"wv{t}")
                nc.sync.dma_start(wvt[:], wvx_d[P * t:P * t + P, :])
                wv_sb.append(wvt)
            for t in range(3):
                xt = xpool.tile([P, S], BF16, name=f"xt{t}", tag=f"xt{t}")
                nc.sync.dma_start(xt[:], xt_d[P * t:P * t + P, :])
                xt_sb.append(xt)
            for h in range(H):
                wot = wpool.tile([97, D], BF16, name=f"wo{h}", tag=f"wo{h}")
                nc.sync.dma_start(wot[:], wo_d[97 * h:97 * h + 97, :])
                wo_sb.append(wot)
            bq_sb = wpool.tile([HD, H], F32, name="bq", tag="bq")
            nc.sync.dma_start(bq_sb[:], bqh_d[:, :])
            bk_sb = wpool.tile([HD, H], F32, name="bk", tag="bk")
            nc.sync.dma_start(bk_sb[:], bkh_d[:, :])
            msk_sb = wpool.tile([P, P], BF16, name="msk", tag="msk")
            nc.sync.dma_start(msk_sb[:], msk_d[:, :])
            mskp_sb = wpool.tile([P, 2, 256], F8, name="mskp", tag="mskp")
            nc.sync.dma_start(mskp_sb[:, :, :], mskp_d[:, :, :])
            sh_sb = wpool.tile([P, 1], F32, name="shift", tag="shift")
            nc.vector.memset(sh_sb[:], -SHIFT)

            # broadcast tiles (host-built): V'-bias+ones rows, output bias
            vb_bc = wpool.tile([P, 97 * H + 31], F32, name="vbbc", tag="vbbc")
            nc.sync.dma_start(vb_bc[:], vb_d[:, :])
            bo_bc = wpool.tile([P, D], F32, name="bobc", tag="bobc")
            nc.sync.dma_start(bo_bc[:], bo_d[:, :])
            ones_sb = wpool.tile([1, P], BF16, name="ones", tag="ones")
            nc.sync.dma_start(ones_sb[:], ones_d[:, :])

            qt_sb, kt_sb = [], []
            for h in range(H):
                qt = qkt_pool.tile([HD, S], BF16, name=f"qt{h}", tag=f"qt{h}")
                qt_sb.append(qt)
                kt = qkt_pool.tile([HD, S], BF16, name=f"kt{h}", tag=f"kt{h}")
                kt_sb.append(kt)
            vall8 = vpool.tile([P, KTN, 4 * P], F8, name="vall8", tag="vall8")
            vallb = vpool.tile([P, 4, 97 * H], BF16, name="vallb", tag="vallb")

            def proj_units(ci):
                """Projection work units for chunk ci: 8 Q/K + 4 V' closures.
                Each emits its matmuls + one DVE drain (bias folds, bf16)."""
                units = []

                def qk_unit(w_sb, b_sb, dst, h):
                    def emit():
                        ps = fpool.tile([HD, CH], F32, name="projps", tag="f")
                        for t in range(3):
                            nc.tensor.matmul(
                                ps[:],
                                w_sb[t][:, HD * h:HD * h + HD],
                                xt_sb[t][:, CH * ci:CH * ci + CH],
                                start=(t == 0), stop=(t == 2))
                        if "qdrainact" in variant and dst is qt_sb:
                            nc.scalar.add(
                                dst[h][:, CH * ci:CH * ci + CH], ps[:], b_sb[:, h:h + 1])
                        else:
                            nc.vector.tensor_scalar_add(
                                dst[h][:, CH * ci:CH * ci + CH], ps[:], b_sb[:, h:h + 1])
                    return emit

                def v_unit(st):
                    def emit():
                        ps = fpool.tile([P, 97 * H + 31], F32, name="vps", tag="f")
                        for t in range(3):
                            nc.tensor.matmul(ps[:],
                                             xt_sb[t][:, P * st:P * st + P],
                                             wv_sb[t][:], start=(t == 0), stop=(t == 2))
                        # fp8 V': head blocks padded to 128 cols; pads carry
                        # junk that only lands in unread acc rows 97..127
                        vdst = vall8[:, st, 0:1]
                        dst_ap = bass.AP(vdst.tensor, vdst.offset,
                                         [list(vdst.ap[0]), [P, H], [1, P]])
                        psb = ps[:, 0:1]
                        src_ap = bass.AP(psb.tensor, psb.offset,
                                         [list(psb.ap[0]), [97, H], [1, P]])
                        vbb = vb_bc[:, 0:1]
                        vb_ap = bass.AP(vbb.tensor, vbb.offset,
                                        [list(vbb.ap[0]), [97, H], [1, P]])
                        nc.vector.tensor_tensor(dst_ap, src_ap, vb_ap, op=add)
                        if st < 4:
                            nc.vector.tensor_tensor(
                                vallb[:, st, :], ps[:, 0:97 * H],
                                vb_bc[:, 0:97 * H], op=add)
                    return emit

                for h in range(H):
                    units.append(qk_unit(wq_sb, bq_sb, qt_sb, h))
                    units.append(qk_unit(wk_sb, bk_sb, kt_sb, h))
                for st in range(4 * ci, 4 * ci + 4):
                    units.append(v_unit(st))
                return units

            def outproj_units(ci, on_tiles):
                units = []

                def o_unit(sj):
                    def emit():
                        st = 4 * ci + sj
                        fo = fpool.tile([P, D], F32, name="fo", tag="f")
                        for h in range(H):
                            nc.tensor.matmul(fo[:], on_tiles[h][:, P * sj:P * sj + P],
                                             wo_sb[h][:], start=(h == 0), stop=(h == 3))
                        fs = onpool.tile([P, D], F32, name="fs", tag="fs", bufs=3)
                        nc.vector.tensor_tensor(fs[:], fo[:], bo_bc[:], op=add)
                        nc.sync.dma_start(out_d[P * st:P * st + P, :], fs[:])
                    return emit

                for sj in range(4):
                    units.append(o_unit(sj))
                return units

            def attend_chunk(ci, filler):
                """Attention for chunk ci; between pair iterations, emit
                independent filler units (next chunk's projections, previous
                chunk's output projection) to keep PE fed during exp waits."""
                on_tiles = []
                nkt = 4 * (ci + 1)
                npairs = H * (nkt // 2)
                fill_every = max(1, npairs // max(1, len(filler)))
                fi = 0
                pair_no = 0
                fp8 = ci >= 1
                PT_DT = F8 if fp8 else BF16

                def emit_pair(h, acc, pr):
                    qk = qkpool.tile([P, 2, CH], F32, name="qk", tag="qk")
                    pt = ppool.tile([P, 2, CH], PT_DT, name="pt", tag="pt")
                    kt0 = 2 * pr
                    rt0 = P * kt0 - CH * ci
                    # pair-level start column (both tiles computed from it
                    # so the pair shares one exp instruction)
                    sc = max(rt0, 0)
                    for j in range(2):
                        nc.tensor.matmul(
                            qk[:, j, sc:CH],
                            kt_sb[h][:, P * (kt0 + j):P * (kt0 + j) + P],
                            qt_sb[h][:, CH * ci + sc:CH * (ci + 1)],
                            start=True, stop=True)
                    if fp8:
                        nc.scalar.activation(pt[:, :, sc:CH], qk[:, :, sc:CH],
                                             Exp, scale=float(SCALE),
                                             bias=sh_sb[:])
                    else:
                        nc.scalar.activation(pt[:, :, sc:CH], qk[:, :, sc:CH],
                                             Exp, scale=float(SCALE))
                    if rt0 >= 0 and fp8:
                        # diagonal pair: one composite-mask op covering
                        # [rt0, rt0+256) of both slabs — triangle+ones on
                        # slab 0, zeros+triangle on slab 1 (the zeros also
                        # clear slab 1's not-yet-valid columns for DR)
                        nc.vector.tensor_tensor(
                            pt[:, :, rt0:rt0 + 256],
                            pt[:, :, rt0:rt0 + 256],
                            mskp_sb[:, :, :], op=mult)
                    elif rt0 >= 0:
                        # diagonal pair: zero both upper triangles in one
                        # strided-AP op; mask broadcast along the pair dim
                        base = pt[:, 0, 0:1]
                        diag_view = bass.AP(
                            base.tensor, base.offset + rt0,
                            [[2 * CH, P], [CH + P, 2], [1, P]])
                        mbc = msk_sb[:].unsqueeze(1).broadcast_to([P, 2, P])
                        nc.vector.tensor_tensor(diag_view, diag_view,
                                                mbc, op=mult)
                    if fp8:
                        nc.tensor.matmul(
                            acc[:, sc:CH],
                            vall8[:, kt0:kt0 + 2, P * h:P * h + P],
                            pt[:, :, sc:CH],
                            start=(pr == 0), stop=(pr == nkt // 2 - 1),
                            skip_group_check=True,
                            perf_mode=mybir.MatmulPerfMode.DoubleRow)
                    else:
                        for j in range(2):
                            kt = kt0 + j
                            scol = max(P * kt - CH * ci, 0)
                            nc.tensor.matmul(
                                acc[0:97, scol:CH],
                                vallb[:, kt, 97 * h:97 * h + 97],
                                pt[:, j, scol:CH],
                                start=(kt == 0), stop=(kt == nkt - 1),
                                skip_group_check=True)

                for hh in range(0, H, 2):
                    acc0 = accpool.tile([P, CH], F32, name="acc", tag="acc")
                    acc1 = accpool.tile([P, CH], F32, name="acc", tag="acc")
                    for pr in range(nkt // 2):
                        emit_pair(hh, acc0, pr)
                        emit_pair(hh + 1, acc1, pr)
                        pair_no += 2
                        if pair_no % fill_every < 2 and fi < len(filler):
                            filler[fi]()
                            fi += 1
                    # normalize the head pair: row 0 of each acc is the
                    # softmax denominator; recips share one tile so the
                    # partition broadcast runs once per two heads
                    rec2 = rpool.tile([1, 2 * CH], F32, name="rec", tag="rec")
                    nc.vector.reciprocal_approx_fast(
                        out=rec2[0:1, 0:CH], in_=acc0[0:1, :])
                    nc.vector.reciprocal_approx_fast(
                        out=rec2[0:1, CH:2 * CH], in_=acc1[0:1, :])
                    rb2 = rpool.tile([97, 2 * CH], F32, name="rb", tag="rb")
                    nc.gpsimd.partition_broadcast(rb2[:], rec2[:], channels=97)
                    for kk, a in ((0, acc0), (1, acc1)):
                        on = onpool.tile([97, CH], BF16, name=f"on{hh + kk}",
                                         tag=f"on{hh + kk}")
                        nc.vector.tensor_tensor(
                            on[:], a[0:97, :], rb2[:, CH * kk:CH * kk + CH], op=mult)
                        on_tiles.append(on)
                while fi < len(filler):
                    filler[fi]()
                    fi += 1
                return on_tiles

            # software pipeline across chunks: during attention of chunk ci,
            # emit chunk ci+1's projections and chunk ci-1's output projection
            for u in proj_units(0):
                u()
            pending_out = []
            for ci in range(NCH):
                filler = list(pending_out)
                if ci + 1 < NCH:
                    filler += proj_units(ci + 1)
                on_tiles = attend_chunk(ci, filler)
                pending_out = outproj_units(ci, on_tiles)
            for u in pending_out:
                u()

        for pool in (fpool, accpool, qkpool, rpool, onpool, ppool, vpool,
                     qkt_pool, xpool, wpool):
            pool.release()

    nc.finalize()
    return nc


_NC_CACHE = None


def get_nc():
    global _NC_CACHE
    if _NC_CACHE is None:
        _NC_CACHE = build_nc()
    return _NC_CACHE


def host_prep(x, Wq, bq, Wk, bk, Wv, bv, Wo, bo):
    """Build per-core input maps (layout prep only; all FLOPs run on device)."""
    x = np.ascontiguousarray(np.asarray(x, dtype=np.float32))
    Wq = np.ascontiguousarray(np.asarray(Wq, dtype=np.float32))
    Wk = np.ascontiguousarray(np.asarray(Wk, dtype=np.float32))
    Wv = np.ascontiguousarray(np.asarray(Wv, dtype=np.float32))
    Wo = np.ascontiguousarray(np.asarray(Wo, dtype=np.float32))
    bq = np.asarray(bq, dtype=np.float32)
    bk = np.asarray(bk, dtype=np.float32)
    bv = np.asarray(bv, dtype=np.float32)
    bo = np.asarray(bo, dtype=np.float32)

    # V' weights: per head block of 97 cols, col 0 reserved for the ones
    # column (weights zero there; the 1.0 comes from the vb row fold)
    wvx = np.zeros((D, 97 * H + 31), np.float32)
    vb = np.zeros((1, 97 * H), np.float32)
    for h in range(H):
        wvx[:, 97 * h + 1:97 * h + 97] = Wv[:, HD * h:HD * h + HD]
        vb[0, 97 * h] = 1.0
        vb[0, 97 * h + 1:97 * h + 97] = bv[HD * h:HD * h + HD]
    vb = np.concatenate([vb, np.zeros((1, 31), np.float32)], axis=1)
    vb = np.ascontiguousarray(np.broadcast_to(vb, (P, 97 * H + 31)))

    jj = np.arange(P)[None, :]
    pp = np.arange(P)[:, None]
    msk = (jj >= pp).astype(ml_dtypes.bfloat16)
    tri = (jj >= pp).astype(np.float32)
    mskp = np.zeros((P, 2, 256), np.float32)
    mskp[:, 0, 0:P] = tri
    mskp[:, 0, P:] = 1.0
    mskp[:, 1, P:] = tri
    mskp = mskp.astype(ml_dtypes.float8_e4m3)

    bqh = np.ascontiguousarray(bq.reshape(H, HD).T)
    bkh = np.ascontiguousarray(bk.reshape(H, HD).T)
    wox = np.zeros((97 * H, D), np.float32)
    for h in range(H):
        wox[97 * h + 1:97 * h + 97, :] = Wo[HD * h:HD * h + HD, :]
    BF = ml_dtypes.bfloat16
    common = dict(wq=Wq.astype(BF), wk=Wk.astype(BF), wvx=wvx.astype(BF),
                  wo=wox.astype(BF), bqh=bqh, bkh=bkh,
                  vb=vb, msk=msk, mskp=mskp,
                  bo=np.ascontiguousarray(np.broadcast_to(bo.reshape(1, D), (P, D))),
                  onesrow=np.ones((1, P), ml_dtypes.bfloat16))
    return [dict(xt=np.ascontiguousarray(x[b].T).astype(ml_dtypes.bfloat16), **common)
            for b in range(x.shape[0])]


def kernel(**inputs):
    in_maps = host_prep(**inputs)
    nc = get_nc()
    res = run_bass_kernel_spmd(nc, in_maps, core_ids=list(range(N_CORES)))
    return np.stack([res.results[b]["out"] for b in range(N_CORES)], axis=0)

